# revision 5
# baseline (speedup 1.0000x reference)
"""Dale-constrained integrator on 8 trn2 NeuronCores.

Strategy: data-parallel over batch (16 rows/core, W replicated), with the
per-step matmul restructured around PE column-tiling:

  s_{t+1} = relu((s_t + e_t) @ M),  e_t folded as X_t @ G (G = E@M on host)

Per step the full [2048, 2048] W must stream through the PE as the moving
operand (stationary = state k-tiles [128, 16]).  The baseline issued the
4 output chunks of 512 sequentially (32768 PE cycles/step).  Here the 4
chunks run CONCURRENTLY in the four 32-column groups of the PE array via
tile_position=(0, 32g): round r issues 4 matmuls sharing stationary
s-tile r, each streaming a different 512-wide W chunk, with outputs
landing on disjoint partition ranges [32g, 32g+16) of ONE psum bank.
Accumulation groups are per-element (has_written), so the four groups
accumulate independently.  ~4x less PE wall time per step.

All matmul operands are bf16 (fp32 psum accumulate): halves SBUF/DMA and
doubles DVE transpose throughput.  The output dots o_r[t] = dec_r . s'
moved from DVE (scalar_tensor_tensor, was 5.4us/step) to the PE: 16 tiny
matmuls contract dec tiles [128, 2] against the freshly transposed state
slabs [128, 16], accumulating o_psum [2, 16].
"""
import sys
sys.path.insert(0, "/opt/trn_rl_repo")
import numpy as np
import ml_dtypes
import concourse.bass as bass
import concourse.tile as tile
from concourse import bacc, mybir
from concourse.bass_utils import run_bass_kernel_spmd

N = 2048          # recurrent units
B = 16            # batch per core
NCORES = 8
T = 1024          # timesteps
NK = 16           # k-tiles of 128
CH = 512          # output chunk = one col-group's moving width
NG = 4            # concurrent PE column groups
RING = 64         # IO ring (DMA in/out every RING steps)

F32 = mybir.dt.float32
BF16 = mybir.dt.bfloat16
AF = mybir.ActivationFunctionType
BF = ml_dtypes.bfloat16

_cached_nc = None


def _build():
    nc = bacc.Bacc("TRN2", target_bir_lowering=False, debug=False)

    W_d = nc.dram_tensor("W", [128, NK * N], BF16, kind="ExternalInput")
    G_d = nc.dram_tensor("G", [2, N], BF16, kind="ExternalInput")
    s0_d = nc.dram_tensor("s0T", [128, NK * B], BF16, kind="ExternalInput")
    xT_d = nc.dram_tensor("xT", [2, T * B], BF16, kind="ExternalInput")
    decT_d = nc.dram_tensor("decT", [128, NK * 2], BF16, kind="ExternalInput")
    o_d = nc.dram_tensor("o01", [2, T * B], F32, kind="ExternalOutput")

    with tile.TileContext(nc) as tc:
        with (
            tc.tile_pool(name="const", bufs=1) as cpool,
            tc.tile_pool(name="state", bufs=1) as spool,
            tc.tile_pool(name="work", bufs=2) as wpool,
            tc.tile_pool(name="oring", bufs=2) as opool,
            tc.tile_pool(name="psum", bufs=2, space="PSUM") as psum,
            tc.tile_pool(name="opsum", bufs=2, space="PSUM") as opsum,
        ):
            W_sb = cpool.tile([128, NK * N], BF16, tag="W")
            for kt in range(NK):
                nc.sync.dma_start(W_sb[:, kt * N:(kt + 1) * N],
                                  W_d[:, kt * N:(kt + 1) * N])
            G_sb = cpool.tile([2, N], BF16, tag="G")
            nc.sync.dma_start(G_sb[:], G_d[:])
            decT = cpool.tile([128, NK * 2], BF16, tag="decT")
            nc.sync.dma_start(decT[:], decT_d[:])

            sT_ab = [spool.tile([128, NK * B], BF16, tag=f"sT{i}",
                                name=f"sT{i}")
                     for i in range(2)]
            nc.sync.dma_start(sT_ab[0][:], s0_d[:])
            s32 = spool.tile([32, N], BF16, tag="s32")
            nc.vector.memset(s32[:], 0.0)

            xslab = opool.tile([2, RING * B], BF16, tag="xslab",
                               name="xslab")
            nc.sync.dma_start(xslab[:], xT_d[:, 0:RING * B])

            for t in range(T):
                sin = sT_ab[t % 2]
                sout = sT_ab[(t + 1) % 2]
                oslot = t % RING
                if oslot == 0:
                    o_ring = opool.tile([2, RING * B], F32, tag="o_ring")
                    cur_x = xslab
                    if t + RING < T:
                        xslab = opool.tile([2, RING * B], BF16,
                                           tag="xslab", name="xslab")
                        nc.sync.dma_start(
                            xslab[:],
                            xT_d[:, (t + RING) * B:(t + 2 * RING) * B])

                acc = psum.tile([128, CH], F32, tag="acc")
                for g in range(NG):
                    nc.tensor.matmul(acc[32 * g:32 * g + B, :],
                                     cur_x[:, oslot * B:(oslot + 1) * B],
                                     G_sb[:, g * CH:(g + 1) * CH],
                                     start=True, stop=False,
                                     tile_position=(0, 32 * g))
                for r in range(NK):
                    last = (r == NK - 1)
                    for g in range(NG):
                        nc.tensor.matmul(
                            acc[32 * g:32 * g + B, :],
                            sin[:, r * B:(r + 1) * B],
                            W_sb[:, r * N + g * CH: r * N + (g + 1) * CH],
                            start=False, stop=last,
                            tile_position=(0, 32 * g))

                for g in range(NG):
                    sl = slice(g * CH, (g + 1) * CH)
                    nc.scalar.activation(s32[0:B, sl],
                                         acc[32 * g:32 * g + B, :], AF.Relu)
                    trq = wpool.tile([32, CH], BF16, tag="trq")
                    nc.vector.transpose(trq[:], s32[:, sl])
                    for r2 in range(4):
                        dst = sout[:].rearrange(
                            "p (kt b) -> p kt b", b=B
                        )[32 * r2:32 * (r2 + 1), 4 * g:4 * g + 4, :]
                        src = trq[:].rearrange(
                            "p (tl b32) -> p tl b32", tl=4
                        )[0:32, :, 32 * r2:32 * r2 + B]
                        nc.vector.tensor_copy(dst, src)

                o_ps = opsum.tile([2, B], F32, tag="ops")
                for r in range(NK):
                    nc.tensor.matmul(o_ps[:], decT[:, 2 * r:2 * r + 2],
                                     sout[:, r * B:(r + 1) * B],
                                     start=(r == 0), stop=(r == NK - 1))
                nc.vector.tensor_copy(o_ring[:, oslot * B:(oslot + 1) * B],
                                      o_ps[:])
                if oslot == RING - 1:
                    t0 = t - RING + 1
                    nc.sync.dma_start(o_d[:, t0 * B:(t0 + RING) * B],
                                      o_ring[:])
    nc.compile()
    return nc


def _prep_in_maps(x0, x1, enc0, enc1, dec0, dec1, W, signs, mask, state0):
    x0 = np.asarray(x0, np.float32)
    x1 = np.asarray(x1, np.float32)
    enc0 = np.asarray(enc0, np.float32)
    enc1 = np.asarray(enc1, np.float32)
    dec0 = np.asarray(dec0, np.float32)
    dec1 = np.asarray(dec1, np.float32)
    W = np.asarray(W, np.float32)
    signs = np.asarray(signs, np.float32)
    mask = np.asarray(mask, np.float32)
    state0 = np.asarray(state0, np.float32)

    # host-side constant prep (layout only + the rank-2 fold G = E @ M)
    M2 = (W * signs[None, :]).T * mask[None, :]                # [k, j]
    E = np.stack([enc0 * mask, enc1 * mask]).astype(np.float64)
    G = (E @ M2.astype(np.float64)).astype(BF)
    W_host = np.ascontiguousarray(
        M2.reshape(NK, 128, N).transpose(1, 0, 2).reshape(128, NK * N)
    ).astype(BF)
    decT = np.empty((128, NK * 2), np.float32)
    for kt in range(NK):
        decT[:, 2 * kt] = dec0[kt * 128:(kt + 1) * 128]
        decT[:, 2 * kt + 1] = dec1[kt * 128:(kt + 1) * 128]
    decT = decT.astype(BF)
    s0T = np.broadcast_to(
        state0.reshape(NK, 128)[:, :, None], (NK, 128, B)
    ).transpose(1, 0, 2).reshape(128, NK * B).astype(BF).copy()
    shared = {"W": W_host, "G": G, "decT": decT, "s0T": s0T}

    in_maps = []
    for c in range(NCORES):
        sl = slice(c * B, (c + 1) * B)
        xT = np.empty((2, T * B), np.float32)
        xT[0] = x0[sl].T.reshape(-1)       # t-major [T*B]
        xT[1] = x1[sl].T.reshape(-1)
        in_maps.append(dict(shared, xT=xT.astype(BF)))
    return in_maps


def kernel(x0, x1, enc0, enc1, dec0, dec1, W, signs, mask, state0):
    global _cached_nc
    in_maps = _prep_in_maps(x0, x1, enc0, enc1, dec0, dec1, W, signs,
                            mask, state0)
    if _cached_nc is None:
        _cached_nc = _build()
    res = run_bass_kernel_spmd(_cached_nc, in_maps,
                               core_ids=list(range(NCORES)))
    outs = []
    for r in res.results:
        o = np.asarray(r["o01"], np.float32).reshape(2, T, B)
        outs.append(o.transpose(0, 2, 1))          # [2, B, T]
    o0 = np.concatenate([o[0] for o in outs], axis=0)
    o1 = np.concatenate([o[1] for o in outs], axis=0)
    return (np.ascontiguousarray(o0, dtype=np.float32),
            np.ascontiguousarray(o1, dtype=np.float32))
